# revision 5
# baseline (speedup 1.0000x reference)
"""Trainium2 Bass kernel for nn_LocallyDense (grouped gather + per-group Dense
+ LeakyReLU + BatchNorm inference).

Sharding: expert-parallel over the 41 groups across 8 cores (6 groups on
core 0, 5 on cores 1-7, padded to 6 with a duplicate so one SPMD program
fits all).

The gather (x columns per group) and all BN constant math happen on the
HOST during input prep — the device program is a pure streamed GEMM in the
transposed formulation out^T[o, b] = W^T x^T:
  - lhsT (stationary) = W K-tile  [K=128, M=128 output-half]
  - rhs  (moving)     = gathered-x K-tile [K=128, N=256 batch]
  - PSUM accumulates 12 K-tiles per (group, output-half)
Per group, gathered-x and W K-tiles are interleaved host-side into one
combined DRAM tensor so a single 1.5 MB dma_start feeds both operands;
loads alternate between the two HWDGE queues (Sync / Scalar) to hide
descriptor-generation gaps. Epilogue is ACT Prelu (bias via per-partition
scalar AP, the transposed layout puts output features on partitions) then
one DVE tensor_scalar for the BN affine, with inv = gamma/sqrt(var+eps)
and c = beta - mean*inv precomputed on host. Output is stored bf16 and
cast/transposed back on the host during unshard.
"""

import numpy as np
import ml_dtypes

B, D_IN, N_GROUPS, G, D_OUT = 256, 65536, 41, 1536, 256
BN_EPS = 1e-3
ALPHA = 0.3
N_CORES = 8
NG = 6                # groups per core (padded)
KT = G // 128         # 12 K-tiles per group
CW = B + D_OUT        # combined tile width per K-tile (x cols + w cols)

USE_BF16 = True       # x/W feed the PE in bf16 (fp32 accumulate in PSUM)
TRACE = False         # set by test.py for profiling runs
TRACE_KW = {}
REPEAT = 1            # run the main loop R times (benchmarking differential)

_prog_cache = {}


def _np_dtx():
    return ml_dtypes.bfloat16 if USE_BF16 else np.float32


def _build_program(use_bf16: bool):
    import concourse.bacc as bacc
    import concourse.mybir as mybir
    import concourse.tile as tile

    f32 = mybir.dt.float32
    dt_x = mybir.dt.bfloat16 if use_bf16 else mybir.dt.float32

    nc = bacc.Bacc("TRN2", target_bir_lowering=False, debug=False,
                   num_devices=N_CORES)
    xw = nc.dram_tensor("xw", [128, NG * KT * CW], dt_x, kind="ExternalInput")
    # cols 0-11: bias[g, h*128+p]; 12-13: inv[h*128+p]; 14-15: c[h*128+p]
    cons = nc.dram_tensor("cons", [128, 16], f32, kind="ExternalInput")
    out = nc.dram_tensor("out", [NG * 2 * 128, B], dt_x, kind="ExternalOutput")

    hwdge = None  # set inside context

    with tile.TileContext(nc) as tc:
        with tc.tile_pool(name="const", bufs=1) as cpool, \
             tc.tile_pool(name="xw", bufs=NG) as xwpool, \
             tc.tile_pool(name="ep", bufs=4) as epool, \
             tc.tile_pool(name="ps", bufs=3, space="PSUM") as ppool:

            hwdge = (nc.sync, nc.scalar)

            ct = cpool.tile([128, 16], f32)
            nc.sync.dma_start(out=ct[:], in_=cons[:, :])

            for g_rep in range(REPEAT * NG):
                g = g_rep % NG
                xwt = xwpool.tile([128, KT, CW], dt_x, tag="xw")
                hwdge[g_rep % 2].dma_start(
                    out=xwt[:], in_=xw[:, g * KT * CW:(g + 1) * KT * CW])
                for h in range(2):
                    ps = ppool.tile([128, B], f32, tag=f"ps{h}",
                                    name=f"ps{h}_{g_rep}")
                    for blk in range(KT):
                        nc.tensor.matmul(
                            out=ps[:],
                            lhsT=xwt[:, blk, B + h * 128:B + (h + 1) * 128],
                            rhs=xwt[:, blk, 0:B],
                            start=(blk == 0), stop=(blk == KT - 1))
                    t = epool.tile([128, B], f32, tag="t")
                    nc.scalar.activation(
                        out=t[:], in_=ps[:],
                        func=mybir.ActivationFunctionType.Prelu,
                        bias=ct[:, 2 * g + h:2 * g + h + 1],
                        scale=1.0, alpha=float(ALPHA))
                    y = epool.tile([128, B], dt_x, tag="y")
                    nc.vector.tensor_scalar(
                        out=y[:], in0=t[:],
                        scalar1=ct[:, 12 + h:13 + h],
                        scalar2=ct[:, 14 + h:15 + h],
                        op0=mybir.AluOpType.mult,
                        op1=mybir.AluOpType.add)
                    hwdge[(g_rep + 1) % 2].dma_start(
                        out=out[(g * 2 + h) * 128:(g * 2 + h + 1) * 128, :],
                        in_=y[:])
    nc.compile()
    return nc


def _get_program(use_bf16: bool):
    key = (use_bf16, REPEAT)
    if key not in _prog_cache:
        _prog_cache[key] = _build_program(use_bf16)
    return _prog_cache[key]


def _prep_inputs(x, gidx, W, b, gamma, beta, mmean, mvar):
    dtx = _np_dtx()
    assign = [list(range(0, 6))] + \
             [list(range(6 + 5 * i, 6 + 5 * (i + 1))) for i in range(7)]
    inv = (gamma.astype(np.float64) /
           np.sqrt(mvar.astype(np.float64) + BN_EPS)).astype(np.float32)
    cvec = (beta - mmean * inv).astype(np.float32)
    inv_pc = inv.reshape(2, 128).T      # [128, 2]
    c_pc = cvec.reshape(2, 128).T       # [128, 2]
    in_maps, metas = [], []
    for c in range(N_CORES):
        gs = assign[c]
        real = len(gs)
        gs6 = gs + [gs[-1]] * (NG - real)
        gi = gidx[gs6]                                   # [NG, G]
        cols = gi.reshape(-1)                            # [NG*G]
        A = x[:, cols]                                   # [B, NG*G] gather
        xw = np.empty((128, NG, KT, CW), dtype=dtx)
        xw[:, :, :, :B] = A.T.reshape(NG, KT, 128, B).transpose(2, 0, 1, 3)
        xw[:, :, :, B:] = W[gs6].reshape(NG, KT, 128, D_OUT) \
                               .transpose(2, 0, 1, 3)
        cons = np.zeros((128, 16), np.float32)
        cons[:, 0:12] = b[gs6].reshape(NG, 2, 128).transpose(2, 0, 1) \
                              .reshape(128, 12)
        cons[:, 12:14] = inv_pc
        cons[:, 14:16] = c_pc
        in_maps.append({"xw": xw.reshape(128, NG * KT * CW),
                        "cons": np.ascontiguousarray(cons)})
        metas.append((gs, real))
    return in_maps, metas


def kernel(**inputs):
    x = np.asarray(inputs["x"], dtype=np.float32)
    gidx = np.asarray(inputs["group_idx"]).astype(np.int64)
    W = np.asarray(inputs["W"], dtype=np.float32)
    b = np.asarray(inputs["b"], dtype=np.float32)
    gamma = np.asarray(inputs["gamma"], dtype=np.float32)
    beta = np.asarray(inputs["beta"], dtype=np.float32)
    mmean = np.asarray(inputs["moving_mean"], dtype=np.float32)
    mvar = np.asarray(inputs["moving_var"], dtype=np.float32)

    in_maps, metas = _prep_inputs(x, gidx, W, b, gamma, beta, mmean, mvar)
    nc = _get_program(USE_BF16)

    from concourse import bass_utils
    res = bass_utils.run_bass_kernel_spmd(
        nc, in_maps, core_ids=list(range(N_CORES)), trace=TRACE, **TRACE_KW)
    if TRACE:
        kernel.last_result = res

    full = np.empty((B, N_GROUPS, D_OUT), dtype=np.float32)
    for c, (gs, real) in enumerate(metas):
        o = res.results[c]["out"].astype(np.float32) \
               .reshape(NG, 2, 128, B)                  # [g, h, p, b]
        oc = o.transpose(3, 0, 1, 2).reshape(B, NG, D_OUT)  # [b, g, o]
        full[:, gs, :] = oc[:, :real, :]
    return full


def host_check():
    """Validate host prep + unshard logic with a numpy matmul (no device)."""
    d = np.load("/root/problem/_ref_cache.npz")
    x = d["x"].astype(np.float32)
    gidx = d["group_idx"].astype(np.int64)
    W, b = d["W"].astype(np.float32), d["b"].astype(np.float32)
    expected = d["expected"]
    in_maps, metas = _prep_inputs(
        x, gidx, W, b, d["gamma"].astype(np.float32),
        d["beta"].astype(np.float32), d["moving_mean"].astype(np.float32),
        d["moving_var"].astype(np.float32))
    full = np.empty((B, N_GROUPS, D_OUT), dtype=np.float32)
    for c, (gs, real) in enumerate(metas):
        m = in_maps[c]
        xw = m["xw"].astype(np.float32).reshape(128, NG, KT, CW)
        cons = m["cons"]
        o = np.empty((NG, 2, 128, B), np.float32)
        for g in range(NG):
            for h in range(2):
                ps = np.zeros((128, B), np.float32)
                for blk in range(KT):
                    ps += (xw[:, g, blk, B + h * 128:B + (h + 1) * 128].T
                           @ xw[:, g, blk, 0:B])
                z = ps + cons[:, 2 * g + h:2 * g + h + 1]
                t = np.where(z >= 0, z, ALPHA * z)
                y = t * cons[:, 12 + h:13 + h] + cons[:, 14 + h:15 + h]
                o[g, h] = y.astype(_np_dtx()).astype(np.float32)
        oc = o.transpose(3, 0, 1, 2).reshape(B, NG, D_OUT)
        full[:, gs, :] = oc[:, :len(gs), :]
    err = np.max(np.abs(full - expected)) / (np.max(np.abs(expected)) + 1e-30)
    print(f"host_check max-abs-rel err = {err:.3e}")
    return err


if __name__ == "__main__":
    host_check()
